# revision 3
# baseline (speedup 1.0000x reference)
"""Trainium2 Bass kernel for BotanHadamardTransform: y = x @ H, with
x [4, 4096, 4096] f32 and H [4096, 4096] f32 the normalized Sylvester
Hadamard matrix H_4096 / 64.

Algorithm (decimation-in-time Kronecker split): H_4096 = H_8 (x) H_512,
so for a row v: y[ja*512+jb] = sum_ib u[ja, ib] * (H_512/64)[ib, jb]
where u[ja] = FWHT_8 over the ia axis of v.reshape(8, 512).

Device mapping per core (1/8 of rows; host pre-transposes so the core
sees xT [4096, 2048] with the 4096 k-dim on partitions):
  - input DMA is SWDGE (gpsimd) with f32->bf16 cast, one DMA per ia
    block (1 MiB, 2 KB runs) from a dedicated 16-slot pool so the input
    stream prefetches ~2 windows ahead of compute
  - the 3-stage FWHT_8 butterfly runs on the INPUT side in bf16 on DVE:
    all-SBUF 16-bit packed ops get the DVE 2x_1p perf mode (2 elem/cyc)
  - PE contracts the 512 factor as bf16 matmuls (1 cycle/row); PSUM
    accumulators [128,4,512] f32 = 4 banks, pool bufs=2 double-buffers
  - PSUM eviction (Activation engine) casts to bf16 = the final output
    values; DMA-out bf16 on the two HWDGE rings (host upcasts to f32,
    rel err ~4e-3, well under the 2e-2 gate)
"""
import sys

sys.path.insert(0, "/opt/trn_rl_repo")

import numpy as np

import concourse.bass as bass  # noqa: F401
import concourse.tile as tile
from concourse import bacc, mybir
from concourse.bass_utils import run_bass_kernel_spmd

N_CORES = 8
N = 4096            # hidden dim
ROWS = 4 * 4096     # total rows
RC = ROWS // N_CORES  # columns of xT per core = 2048

B = 512             # PE-contracted Kronecker factor (Hf = H_512/64)
A = N // B          # butterfly factor (8)
R = 512             # moving columns per r-tile window
SUB = B // 128      # 128-chunks per B block (4)


def _build():
    nc = bacc.Bacc("TRN2", target_bir_lowering=False, debug=False,
                   num_devices=N_CORES)
    xT_ap = nc.dram_tensor("xT", [N, RC], mybir.dt.float32,
                           kind="ExternalInput").ap()
    hf_ap = nc.dram_tensor("Hf", [B, B], mybir.dt.float32,
                           kind="ExternalInput").ap()
    yT_ap = nc.dram_tensor("yT", [N, RC], mybir.dt.bfloat16,
                           kind="ExternalOutput").ap()

    f32 = mybir.dt.float32
    bf16 = mybir.dt.bfloat16

    xT_v = xT_ap.rearrange("(ia s p) r -> p ia s r", p=128, s=SUB)
    # output row = ja*512 + q*128 + p  (chunk = ja*4 + q)
    yT_v = yT_ap.rearrange("(ja q p) r -> p ja q r", q=SUB, p=128)

    n_rt = RC // R

    with tile.TileContext(nc) as tc:
        with (
            tc.tile_pool(name="hfp", bufs=1) as hfp,
            tc.tile_pool(name="xin", bufs=16) as xinp,
            tc.tile_pool(name="bf", bufs=12) as bfp,
            tc.tile_pool(name="yq", bufs=5) as yqp,
            tc.tile_pool(name="ps", bufs=2, space="PSUM") as psp,
        ):
            # stationary Hf: DMA f32 staging, one cast copy to bf16.
            hf_st = bfp.tile([128, SUB, B], f32, tag="bf", name="hf_stage")
            for s in range(SUB):
                nc.sync.dma_start(hf_st[:, s, :],
                                  hf_ap[s * 128:(s + 1) * 128, :])
            hf_bf = hfp.tile([128, SUB, B], bf16, tag="hfb")
            nc.scalar.copy(hf_bf[:], hf_st[:])

            def hfb(s, q):
                # lhsT block [k=128 (i_b sub-chunk s), m=128 (j_b sub q)]
                return hf_bf[:, s, q * 128:(q + 1) * 128]

            flat = lambda t: t[:].rearrange("p c r -> p (c r)")
            half = lambda t, h: t[:, 4 * h:4 * h + 4, :].rearrange(
                "p c r -> p (c r)")

            for it in range(n_rt):
                r0 = it * R
                # ---- input: one SWDGE cast-DMA per ia block (1 MiB).
                # order pairs (ia, ia+4) so S1 butterflies start early.
                xin = [None] * A
                for ia in (0, 4, 1, 5, 2, 6, 3, 7):
                    xin[ia] = xinp.tile([128, SUB, R], bf16, tag="xin",
                                        name=f"x_{it}_{ia}")
                    nc.gpsimd.dma_start(
                        xin[ia][:], xT_v[:, ia, :, r0:r0 + R])

                # ---- FWHT_8 over ia, bf16 on DVE (2x_1p mode).
                # S1 (dist 4): t1[m] holds ia {2m, 2m+1}
                t1 = [bfp.tile([128, A, R], bf16, tag="bf",
                               name=f"t1_{it}_{i}") for i in range(4)]
                for ia in range(4):
                    dst, sl = t1[ia // 2], ia % 2
                    nc.vector.tensor_add(half(dst, sl), flat(xin[ia]),
                                         flat(xin[ia + 4]))
                    dst, sl = t1[2 + ia // 2], ia % 2
                    nc.vector.tensor_sub(half(dst, sl), flat(xin[ia]),
                                         flat(xin[ia + 4]))
                # S2 (dist 2): (t1[0],t1[1]) and (t1[2],t1[3])
                t2 = [bfp.tile([128, A, R], bf16, tag="bf",
                               name=f"t2_{it}_{i}") for i in range(4)]
                for (ma, mb) in ((0, 1), (2, 3)):
                    nc.vector.tensor_add(flat(t2[ma]), flat(t1[ma]),
                                         flat(t1[mb]))
                    nc.vector.tensor_sub(flat(t2[mb]), flat(t1[ma]),
                                         flat(t1[mb]))
                # S3 (dist 1): within each t2 block; u[j] holds ja {2j,2j+1}
                u = [bfp.tile([128, A, R], bf16, tag="bf",
                              name=f"u_{it}_{j}") for j in range(4)]
                for j in range(4):
                    nc.vector.tensor_add(half(u[j], 0), half(t2[j], 0),
                                         half(t2[j], 1))
                    nc.vector.tensor_sub(half(u[j], 1), half(t2[j], 0),
                                         half(t2[j], 1))

                # ---- matmuls + eviction, per output column sub-chunk q
                for q in range(SUB):
                    yq = yqp.tile([128, A, R], bf16, tag="yq",
                                  name=f"y_{it}_{q}")
                    for jh in range(2):
                        acc = psp.tile([128, 4, R], f32, tag="ps",
                                       name=f"acc_{it}_{q}_{jh}")
                        for jj in range(4):
                            ja = jh * 4 + jj
                            for s in range(SUB):
                                nc.tensor.matmul(
                                    acc[:, jj, :],
                                    hfb(s, q),
                                    u[ja // 2][:, (ja % 2) * 4 + s, :],
                                    start=(s == 0),
                                    stop=(s == SUB - 1),
                                )
                        # eviction IS the final output write (f32 -> bf16)
                        nc.scalar.copy(
                            yq[:, 4 * jh:4 * jh + 4, :].rearrange(
                                "p c r -> p (c r)"),
                            acc[:].rearrange("p c r -> p (c r)"))
                    eng = nc.sync if q % 2 == 0 else nc.scalar
                    eng.dma_start(yT_v[:, :, q, r0:r0 + R], yq[:])

    nc.compile()
    return nc


_prog = None


def _get_prog():
    global _prog
    if _prog is None:
        _prog = _build()
    return _prog


def _run(xT, Hf, trace=False):
    nc = _get_prog()
    in_maps = [
        {"xT": np.ascontiguousarray(xT[:, c * RC:(c + 1) * RC]), "Hf": Hf}
        for c in range(N_CORES)
    ]
    res = run_bass_kernel_spmd(nc, in_maps, core_ids=list(range(N_CORES)),
                               trace=trace)
    return res


def kernel(x, H):
    x = np.asarray(x)
    H = np.asarray(H)
    xT = np.ascontiguousarray(x.reshape(ROWS, N).T)          # [N, ROWS]
    Hf = np.ascontiguousarray(H[:B, :B])                      # = H_512 / 64
    res = _run(xT, Hf)
    y = np.empty((ROWS, N), dtype=np.float32)
    for c in range(N_CORES):
        y[c * RC:(c + 1) * RC, :] = \
            np.asarray(res.results[c]["yT"]).astype(np.float32).T
    return y.reshape(4, 4096, N)


# revision 5
# speedup vs baseline: 1.0992x; 1.0992x over previous
"""Trainium2 Bass kernel for BotanHadamardTransform: y = x @ H, with
x [4, 4096, 4096] f32 and H [4096, 4096] f32 the normalized Sylvester
Hadamard matrix H_4096 / 64.

Algorithm (decimation-in-time Kronecker split): H_4096 = H_8 (x) H_512,
so for a row v: y[ja*512+jb] = sum_ib u[ja, ib] * (H_512/64)[ib, jb]
where u[ja] = FWHT_8 over the ia axis of v.reshape(8, 512).

Device mapping per core (1/8 of rows; host pre-transposes so the core
sees xT [4096, 2048] with the 4096 k-dim on partitions):
  - input DMA is SWDGE (gpsimd) with f32->bf16 cast, one DMA per ia
    block (1 MiB, 2 KB runs) from a dedicated pool so the input stream
    prefetches ahead of compute; HBM traffic is 32 MiB in + 16 MiB out
    per core (~130 us of DMA-engine time at the measured rates)
  - the 3-stage FWHT_8 butterfly runs on the INPUT side in bf16 on DVE:
    all-SBUF 16-bit packed ops get the DVE 2x_1p perf mode (2 elem/cyc)
  - PE contracts the 512 factor as bf16 matmuls (1 cycle/row); PSUM
    accumulators [128,4,512] f32 = 4 banks, pool bufs=2 double-buffers
  - PSUM eviction (Activation engine) casts to bf16 = the final output
    values, written into buffers spanning a PAIR of 512-col windows so
    the bf16 DMA-out runs are 2 KB; outputs alternate the two HWDGE
    rings (host upcasts to f32; rel err ~4e-3, under the 2e-2 gate)
"""
import sys

sys.path.insert(0, "/opt/trn_rl_repo")

import numpy as np

import concourse.bass as bass  # noqa: F401
import concourse.tile as tile
from concourse import bacc, mybir
from concourse.bass_utils import run_bass_kernel_spmd

N_CORES = 8
N = 4096            # hidden dim
ROWS = 4 * 4096     # total rows
RC = ROWS // N_CORES  # columns of xT per core = 2048

B = 512             # PE-contracted Kronecker factor (Hf = H_512/64)
A = N // B          # butterfly factor (8)
R = 512             # moving columns per r-tile window
SUB = B // 128      # 128-chunks per B block (4)


def _build():
    nc = bacc.Bacc("TRN2", target_bir_lowering=False, debug=False,
                   num_devices=N_CORES)
    xT_ap = nc.dram_tensor("xT", [N, RC], mybir.dt.float32,
                           kind="ExternalInput").ap()
    hf_ap = nc.dram_tensor("Hf", [B, B], mybir.dt.float32,
                           kind="ExternalInput").ap()
    yT_ap = nc.dram_tensor("yT", [N, RC], mybir.dt.bfloat16,
                           kind="ExternalOutput").ap()

    f32 = mybir.dt.float32
    bf16 = mybir.dt.bfloat16

    xT_v = xT_ap.rearrange("(ia s p) r -> p ia s r", p=128, s=SUB)
    # output row = ja*512 + q*128 + p  (chunk = ja*4 + q)
    yT_v = yT_ap.rearrange("(ja q p) r -> p ja q r", q=SUB, p=128)

    n_rt = RC // R

    with tile.TileContext(nc) as tc:
        with (
            tc.tile_pool(name="hfp", bufs=1) as hfp,
            tc.tile_pool(name="xin", bufs=10) as xinp,
            tc.tile_pool(name="bf", bufs=12) as bfp,
            tc.tile_pool(name="yq", bufs=8) as yqp,
            tc.tile_pool(name="ps", bufs=2, space="PSUM") as psp,
        ):
            # stationary Hf: DMA f32 staging, one cast copy to bf16.
            hf_st = bfp.tile([128, SUB, B], f32, tag="bf", name="hf_stage")
            for s in range(SUB):
                nc.sync.dma_start(hf_st[:, s, :],
                                  hf_ap[s * 128:(s + 1) * 128, :])
            hf_bf = hfp.tile([128, SUB, B], bf16, tag="hfb")
            nc.scalar.copy(hf_bf[:], hf_st[:])

            def hfb(s, q):
                # lhsT block [k=128 (i_b sub-chunk s), m=128 (j_b sub q)]
                return hf_bf[:, s, q * 128:(q + 1) * 128]

            ypairs = []
            flat = lambda t: t[:].rearrange("p c r -> p (c r)")
            half = lambda t, h: t[:, 4 * h:4 * h + 4, :].rearrange(
                "p c r -> p (c r)")

            for it in range(n_rt):
                r0 = it * R
                # ---- input: one SWDGE cast-DMA per ia block (1 MiB).
                # order pairs (ia, ia+4) so S1 butterflies start early.
                xin = [None] * A
                for ia in (0, 4, 1, 5, 2, 6, 3, 7):
                    xin[ia] = xinp.tile([128, SUB, R], bf16, tag="xin",
                                        name=f"x_{it}_{ia}")
                    nc.gpsimd.dma_start(
                        xin[ia][:], xT_v[:, ia, :, r0:r0 + R])

                # ---- FWHT_8 over ia, bf16 on DVE (2x_1p mode).
                # S1 (dist 4): t1[m] holds ia {2m, 2m+1}
                t1 = [bfp.tile([128, A, R], bf16, tag="bf",
                               name=f"t1_{it}_{i}") for i in range(4)]
                for ia in range(4):
                    dst, sl = t1[ia // 2], ia % 2
                    nc.vector.tensor_add(half(dst, sl), flat(xin[ia]),
                                         flat(xin[ia + 4]))
                    dst, sl = t1[2 + ia // 2], ia % 2
                    nc.vector.tensor_sub(half(dst, sl), flat(xin[ia]),
                                         flat(xin[ia + 4]))
                # S2 (dist 2): (t1[0],t1[1]) and (t1[2],t1[3])
                t2 = [bfp.tile([128, A, R], bf16, tag="bf",
                               name=f"t2_{it}_{i}") for i in range(4)]
                for (ma, mb) in ((0, 1), (2, 3)):
                    nc.vector.tensor_add(flat(t2[ma]), flat(t1[ma]),
                                         flat(t1[mb]))
                    nc.vector.tensor_sub(flat(t2[mb]), flat(t1[ma]),
                                         flat(t1[mb]))
                # S3 (dist 1): within each t2 block; u[j] holds ja {2j,2j+1}
                u = [bfp.tile([128, A, R], bf16, tag="bf",
                              name=f"u_{it}_{j}") for j in range(4)]
                for j in range(4):
                    nc.vector.tensor_add(half(u[j], 0), half(t2[j], 0),
                                         half(t2[j], 1))
                    nc.vector.tensor_sub(half(u[j], 1), half(t2[j], 0),
                                         half(t2[j], 1))

                # ---- matmuls + eviction, per output column sub-chunk q.
                # Output buffers span a PAIR of windows (1024 r-cols) so
                # the bf16 DMA-out runs are 2 KB, not 1 KB: ~40% less DMA
                # engine time on the write stream.
                wpar = it % 2
                if wpar == 0:
                    ypair = [yqp.tile([128, 4, 2 * R], bf16, tag="yq",
                                      name=f"y_{it}_{q}_{jh}")
                             for q in range(SUB) for jh in range(2)]
                    ypairs.append(ypair)
                ypair = ypairs[-1]
                for q in range(SUB):
                    for jh in range(2):
                        acc = psp.tile([128, 4, R], f32, tag="ps",
                                       name=f"acc_{it}_{q}_{jh}")
                        for jj in range(4):
                            ja = jh * 4 + jj
                            for s in range(SUB):
                                nc.tensor.matmul(
                                    acc[:, jj, :],
                                    hfb(s, q),
                                    u[ja // 2][:, (ja % 2) * 4 + s, :],
                                    start=(s == 0),
                                    stop=(s == SUB - 1),
                                )
                        # eviction IS the final output write (f32 -> bf16)
                        yq = ypair[q * 2 + jh]
                        nc.scalar.copy(
                            yq[:, :, wpar * R:(wpar + 1) * R], acc[:])
                        if wpar == 1:
                            eng = nc.sync if (q * 2 + jh) % 2 == 0 \
                                else nc.scalar
                            eng.dma_start(
                                yT_v[:, 4 * jh:4 * jh + 4, q,
                                     r0 - R:r0 + R],
                                yq[:])

    nc.compile()
    return nc


_prog = None


def _get_prog():
    global _prog
    if _prog is None:
        _prog = _build()
    return _prog


def _run(xT, Hf, trace=False):
    nc = _get_prog()
    in_maps = [
        {"xT": np.ascontiguousarray(xT[:, c * RC:(c + 1) * RC]), "Hf": Hf}
        for c in range(N_CORES)
    ]
    res = run_bass_kernel_spmd(nc, in_maps, core_ids=list(range(N_CORES)),
                               trace=trace)
    return res


def kernel(x, H):
    x = np.asarray(x)
    H = np.asarray(H)
    xT = np.ascontiguousarray(x.reshape(ROWS, N).T)          # [N, ROWS]
    Hf = np.ascontiguousarray(H[:B, :B])                      # = H_512 / 64
    res = _run(xT, Hf)
    y = np.empty((ROWS, N), dtype=np.float32)
    for c in range(N_CORES):
        y[c * RC:(c + 1) * RC, :] = \
            np.asarray(res.results[c]["yT"]).astype(np.float32).T
    return y.reshape(4, 4096, N)
